# revision 1
# baseline (speedup 1.0000x reference)
"""CCPL contrastive-loss kernel for Trainium2 (8 NeuronCores).

Strategy: the loss only touches 256 sampled 3x3 neighborhoods of
feat_q/feat_k (~4.7 MB of each 512 MiB tensor), so the kernel never
streams the full tensors.  Work is data-parallel over the batch dim:
core b receives feat_q[b] / feat_k[b] (64 MiB each staged to HBM) and a
program with the 256 sample windows baked in as static strided DMAs
(sample_ids are host-known at build time, identical for every core, so
the program is SPMD-clean).  Each core gathers [64c, 256s, 9] blocks for
q and k, normalizes over the channel dim, and emits one partial
sum(|q_hat - k_hat|); the host sums the 8 partials and divides by the
element count.
"""

import os
import sys
from contextlib import ExitStack

import numpy as np

sys.path.insert(0, "/opt/trn_rl_repo")

import concourse.bass as bass
import concourse.tile as tile
from concourse import mybir
from concourse.bass_utils import run_bass_kernel_spmd


def _install_ntff_hook():
    """Provide antenv.axon_hooks when the agent image lacks it.

    concourse's axon trace path imports antenv.axon_hooks to fetch the
    NTFF profile hook; this image's antenv has no such submodule.  The
    hook implementation ships in trn_agent_boot.trn_boot, so wire it up
    against the axon PJRT .so directly.
    """
    try:
        from antenv.axon_hooks import get_axon_ntff_profile_hook  # noqa: F401

        return
    except ImportError:
        pass
    import types

    hook = None
    try:
        from trn_agent_boot.trn_boot import _ntff_profile_via_ctypes

        so = "/opt/axon/libaxon_pjrt.so"
        if os.path.exists(so):
            hook = _ntff_profile_via_ctypes(so)
    except Exception:
        hook = None
    mod = types.ModuleType("antenv.axon_hooks")
    _state = {"hook": hook}
    mod.get_axon_ntff_profile_hook = lambda: _state["hook"]
    mod.set_axon_ntff_profile_hook = lambda h: _state.update(hook=h)
    import antenv

    sys.modules["antenv.axon_hooks"] = mod
    antenv.axon_hooks = mod


_install_ntff_hook()

B, C, H, W = 8, 64, 512, 512
NUM_S = 256
EPS = 1e-7
NCOL = NUM_S * 9  # 2304 columns: (sample, 3x3 window) with center at j=4
CHUNK = 384  # matmul moving-free <= 512; 6 even chunks
NCHUNK = NCOL // CHUNK
N_CORES = 8

_cache: dict = {}
LAST_RESULTS = None  # BassKernelResults of the most recent run (for test.py)


def _split_multi_waits(nc):
    """Walrus build here embeds at most ONE sync wait per instruction.

    Tile emits instructions (notably the kernel-tail Drain) carrying many
    sem waits.  Hoist all but the last wait of any such instruction onto
    single-wait NOPs inserted immediately before it on the same queue —
    the queue stalls on each NOP in turn, preserving semantics.
    """
    from concourse import mybir as _mybir

    for f in nc.m.functions:
        for blk in f.blocks:
            insts = blk.instructions
            i = 0
            while i < len(insts):
                inst = insts[i]
                si = inst.sync_info
                if si is not None and si.on_wait and len(si.on_wait) > 1:
                    waits = list(si.on_wait)
                    si.on_wait = waits[-1:]
                    for j, w in enumerate(waits[:-1]):
                        nop = _mybir.InstNoOp(
                            name=nc.get_next_instruction_name(),
                            ins=[],
                            outs=[],
                            engine=inst.engine,
                            sync_info=_mybir.SyncInfo(on_wait=[w], on_update=[]),
                        )
                        insts.insert(i + j, nop)
                    i += len(waits) - 1
                i += 1


def _build(ids):
    f32 = mybir.dt.float32
    P = 2 * C  # q on partitions 0-63, k on 64-127
    nc = bass.Bass()
    # q and k stacked: the (tensor, channel) dims merge into one uniform
    # 128-row stride, so a single DMA per sample feeds all 16 SDMA ports.
    fqk = nc.dram_tensor("fqk", [P, H, W], f32, kind="ExternalInput")
    # [I64; -I64] so (q_hat - k_hat) falls out of one K=128 matmul
    wdiff = nc.dram_tensor("wdiff", [P, C], f32, kind="ExternalInput")
    out = nc.dram_tensor("out", [1, 1], f32, kind="ExternalOutput")

    with tile.TileContext(nc) as tc, ExitStack() as ctx:
        sb = ctx.enter_context(tc.tile_pool(name="sb", bufs=1))
        work = ctx.enter_context(tc.tile_pool(name="work", bufs=3))
        pn = ctx.enter_context(tc.tile_pool(name="pn", bufs=1, space="PSUM"))
        pbc = ctx.enter_context(tc.tile_pool(name="pbc", bufs=2, space="PSUM"))
        pd = ctx.enter_context(tc.tile_pool(name="pd", bufs=2, space="PSUM"))
        pf = ctx.enter_context(tc.tile_pool(name="pf", bufs=1, space="PSUM"))

        ones = sb.tile([P, 1], f32)
        nc.vector.memset(ones[:], 1.0)
        ones_row = sb.tile([1, C], f32)
        nc.vector.memset(ones_row[:], 1.0)
        wd = sb.tile([P, C], f32)
        nc.sync.dma_start(out=wd[:], in_=wdiff[:])
        # PE warmup so later matmuls don't pay a fresh DVE-clock wait.
        warm = pf.tile([1, 1], f32, tag="warm")
        nc.tensor.matmul(
            out=warm[:], lhsT=ones[:], rhs=ones[:], start=True, stop=True
        )

        qkraw = sb.tile([P, NUM_S, 9], f32)
        # Gather 3x3 windows: ONE strided DMA per sample covering q and k
        # (12B contiguous runs x 3 rows x 128 stacked channels).  The
        # bottleneck is descriptor generation (~4 ns/descriptor per ring),
        # so spread samples over all three generators: SP and ACT HWDGE
        # rings plus the gpsimd SWDGE ring (a bit slower per descriptor).
        qeng = [
            nc.sync, nc.scalar, nc.gpsimd, nc.sync,
            nc.scalar, nc.sync, nc.scalar, nc.gpsimd,
        ]
        for s, (h, w) in enumerate(ids):
            qeng[s % 8].dma_start(
                out=qkraw[:, s, :], in_=fqk[:, h : h + 3, w : w + 3]
            )

        # Process samples in groups so compute streams behind the gathers.
        GS = 32  # samples per group
        GC = GS * 9  # 288 columns (matmul moving-free <= 512)
        NG = NUM_S // GS
        d = sb.tile([P, NUM_S, 9], f32)
        d2 = sb.tile([P, NUM_S, 9], f32)
        df_ = d[:].rearrange("p s n -> p (s n)")
        d2f = d2[:].rearrange("p s n -> p (s n)")
        # q norms in cols [0, NCOL), k norms in cols [NCOL, 2*NCOL): engine
        # writes must stay at partition base 0
        norm = sb.tile([1, 2 * NCOL], f32)
        rinv = sb.tile([1, 2 * NCOL], f32)
        acc = sb.tile([C, NG], f32)

        for g in range(NG):
            ss = slice(g * GS, (g + 1) * GS)
            sl = slice(g * GC, (g + 1) * GC)
            slk = slice(NCOL + g * GC, NCOL + (g + 1) * GC)
            # d = window - center (center column j=4 becomes exactly 0)
            nc.vector.tensor_tensor(
                out=d[:, ss, :],
                in0=qkraw[:, ss, :],
                in1=qkraw[:, ss, 4:5].to_broadcast([P, GS, 9]),
                op=mybir.AluOpType.subtract,
            )
            nc.scalar.square(out=d2[:, ss, :], in_=d[:, ss, :])
            # norm2[col] = sum_c d2[c, col], q and k halves separately
            n2q = pn.tile([1, GC], f32, tag="n2q")
            n2k = pn.tile([1, GC], f32, tag="n2k")
            nc.tensor.matmul(
                out=n2q[:], lhsT=ones[0:C, :], rhs=d2f[0:C, sl],
                start=True, stop=True,
            )
            nc.tensor.matmul(
                out=n2k[:], lhsT=ones[C:P, :], rhs=d2f[C:P, sl],
                start=True, stop=True,
            )
            nc.scalar.sqrt(out=norm[:, sl], in_=n2q[:])
            nc.scalar.sqrt(out=norm[:, slk], in_=n2k[:])
            # rinv = 1/(sqrt(norm2)+eps); center cols give d*(1/eps) = 0
            nc.vector.tensor_scalar_add(
                out=norm[:, sl], in0=norm[:, sl], scalar1=EPS
            )
            nc.vector.tensor_scalar_add(
                out=norm[:, slk], in0=norm[:, slk], scalar1=EPS
            )
            nc.vector.reciprocal(out=rinv[:, sl], in_=norm[:, sl])
            nc.vector.reciprocal(out=rinv[:, slk], in_=norm[:, slk])
            # two K=1 matmuls broadcast rinv_q/rinv_k onto partition
            # quadrants 0 and 64 of one PSUM tile
            bc = pbc.tile([P, GC], f32)
            nc.tensor.matmul(
                out=bc[0:C, :], lhsT=ones_row[:], rhs=rinv[:, sl],
                start=True, stop=True,
            )
            nc.tensor.matmul(
                out=bc[C:P, :], lhsT=ones_row[:], rhs=rinv[:, slk],
                start=True, stop=True,
            )
            qkh = work.tile([P, GC], f32, tag="qkh")
            nc.vector.tensor_tensor(
                out=qkh[:], in0=df_[:, sl], in1=bc[:], op=mybir.AluOpType.mult
            )
            # q_hat - k_hat across the partition halves via [I; -I] matmul
            dif = pd.tile([C, GC], f32, tag="dif")
            nc.tensor.matmul(
                out=dif[:], lhsT=wd[:], rhs=qkh[:], start=True, stop=True
            )
            nc.vector.tensor_reduce(
                out=acc[:, g : g + 1],
                in_=dif[:],
                axis=mybir.AxisListType.X,
                op=mybir.AluOpType.add,
                apply_absolute_value=True,
            )

        accs = sb.tile([C, 1], f32)
        nc.vector.tensor_reduce(
            out=accs[:], in_=acc[:], axis=mybir.AxisListType.X, op=mybir.AluOpType.add
        )
        pfin = pf.tile([1, 1], f32, tag="fin")
        nc.tensor.matmul(
            out=pfin[:], lhsT=accs[:], rhs=ones[0:C, :], start=True, stop=True
        )
        res = sb.tile([1, 1], f32)
        nc.scalar.copy(out=res[:], in_=pfin[:])
        nc.gpsimd.dma_start(out=out[:], in_=res[:])

    _split_multi_waits(nc)
    return nc


def kernel(feat_q, feat_k, sample_ids, *, trace=False, trace_cores=None):
    global LAST_RESULTS
    feat_q = np.ascontiguousarray(np.asarray(feat_q), dtype=np.float32)
    feat_k = np.ascontiguousarray(np.asarray(feat_k), dtype=np.float32)
    ids = np.asarray(sample_ids)
    ids_key = tuple(map(tuple, ids.astype(np.int64).tolist()))
    if ids_key not in _cache:
        _cache[ids_key] = _build(ids_key)
    nc = _cache[ids_key]

    eye = np.eye(C, dtype=np.float32)
    wd = np.concatenate([eye, -eye], axis=0)  # [128, 64]
    in_maps = [
        {
            "fqk": np.concatenate([feat_q[b], feat_k[b]], axis=0),
            "wdiff": wd,
        }
        for b in range(N_CORES)
    ]
    results = run_bass_kernel_spmd(
        nc,
        in_maps,
        core_ids=list(range(N_CORES)),
        trace=trace,
        trace_cores=trace_cores,
    )
    LAST_RESULTS = results
    total = np.float64(0.0)
    for r in results.results:
        total += np.float64(r["out"][0, 0])
    loss = total / (B * C * 8 * NUM_S)
    return np.asarray(loss, dtype=np.float32)



# revision 2
# speedup vs baseline: 2.7000x; 2.7000x over previous
"""CCPL contrastive-loss kernel for Trainium2 (8 NeuronCores).

Strategy: the loss only touches 256 sampled 3x3 neighborhoods of
feat_q/feat_k (~4.7 MB of each 512 MiB tensor), so the kernel never
streams the full tensors.  Work is data-parallel over the batch dim:
core b receives feat_q[b] / feat_k[b] staged CHANNEL-LAST ([H*W, 128]
with q on c 0-63, k on c 64-127), so each sampled pixel is one 512 B
contiguous run.  One indirect DMA per half (offset-table driven SWDGE
gather, 128-partition x 9-window offsets) pulls the 1.18 MB of touched
data; samples live on partitions so the normalize / L1 math runs as
wide [128, n] vector/scalar ops.  Each core emits one partial
sum(|q_hat - k_hat|); the host sums the 8 partials and divides by the
element count.
"""

import os
import sys
from contextlib import ExitStack

import numpy as np

sys.path.insert(0, "/opt/trn_rl_repo")

import concourse.bass as bass
import concourse.tile as tile
from concourse import mybir
from concourse.bass_utils import run_bass_kernel_spmd


def _install_ntff_hook():
    """Provide antenv.axon_hooks when the agent image lacks it.

    concourse's axon trace path imports antenv.axon_hooks to fetch the
    NTFF profile hook; this image's antenv has no such submodule.  The
    hook implementation ships in trn_agent_boot.trn_boot, so wire it up
    against the axon PJRT .so directly.
    """
    try:
        from antenv.axon_hooks import get_axon_ntff_profile_hook  # noqa: F401

        return
    except ImportError:
        pass
    import types

    hook = None
    try:
        from trn_agent_boot.trn_boot import _ntff_profile_via_ctypes

        so = "/opt/axon/libaxon_pjrt.so"
        if os.path.exists(so):
            hook = _ntff_profile_via_ctypes(so)
    except Exception:
        hook = None
    mod = types.ModuleType("antenv.axon_hooks")
    _state = {"hook": hook}
    mod.get_axon_ntff_profile_hook = lambda: _state["hook"]
    mod.set_axon_ntff_profile_hook = lambda h: _state.update(hook=h)
    import antenv

    sys.modules["antenv.axon_hooks"] = mod
    antenv.axon_hooks = mod


_install_ntff_hook()

B, C, H, W = 8, 64, 512, 512
NUM_S = 256
EPS = 1e-7
P = 2 * C  # q on c 0-63, k on c 64-127 of each pixel's 128-float run
NG = 2  # sample groups (128 samples each): overlap gather g1 with compute g0
SG = NUM_S // NG  # 128 samples per group -> one per partition
N_CORES = 8

_cache: dict = {}
LAST_RESULTS = None  # BassKernelResults of the most recent run (for test.py)


def _split_multi_waits(nc):
    """Walrus build here embeds at most ONE sync wait per instruction.

    Tile emits instructions (notably the kernel-tail Drain) carrying many
    sem waits.  Hoist all but the last wait of any such instruction onto
    single-wait NOPs inserted immediately before it on the same queue —
    the queue stalls on each NOP in turn, preserving semantics.
    """
    from concourse import mybir as _mybir

    for f in nc.m.functions:
        for blk in f.blocks:
            insts = blk.instructions
            i = 0
            while i < len(insts):
                inst = insts[i]
                si = inst.sync_info
                if si is not None and si.on_wait and len(si.on_wait) > 1:
                    waits = list(si.on_wait)
                    si.on_wait = waits[-1:]
                    for j, w in enumerate(waits[:-1]):
                        nop = _mybir.InstNoOp(
                            name=nc.get_next_instruction_name(),
                            ins=[],
                            outs=[],
                            engine=inst.engine,
                            sync_info=_mybir.SyncInfo(on_wait=[w], on_update=[]),
                        )
                        insts.insert(i + j, nop)
                    i += len(waits) - 1
                i += 1


def _build():
    f32 = mybir.dt.float32
    u32 = mybir.dt.uint32
    nc = bass.Bass()
    # channel-last: pixel p of the batch is fqk[p, :] (512 B contiguous)
    fqk = nc.dram_tensor("fqk", [H * W, P], f32, kind="ExternalInput")
    # offs[s_lo, g*9 + rw] = pixel index of window cell rw of sample g*128+s_lo
    offs = nc.dram_tensor("offs", [SG, NG * 9], u32, kind="ExternalInput")
    out = nc.dram_tensor("out", [1, 1], f32, kind="ExternalOutput")

    with tile.TileContext(nc) as tc, ExitStack() as ctx:
        sb = ctx.enter_context(tc.tile_pool(name="sb", bufs=1))
        pf = ctx.enter_context(tc.tile_pool(name="pf", bufs=1, space="PSUM"))

        ones = sb.tile([SG, 1], f32)
        nc.vector.memset(ones[:], 1.0)
        off_t = sb.tile([SG, NG * 9], u32)
        nc.sync.dma_start(out=off_t[:], in_=offs[:])

        # X[s_lo, g, rw, c]: the gathered 3x3 windows, c-minor
        x = sb.tile([SG, NG, 9, P], f32)
        for g in range(NG):
            nc.gpsimd.indirect_dma_start(
                out=x[:, g, :, :],
                out_offset=None,
                in_=fqk[:, :],
                in_offset=bass.IndirectOffsetOnAxis(
                    ap=off_t[:, g * 9 : (g + 1) * 9], axis=0
                ),
            )

        d = sb.tile([SG, NG, 9, P], f32)
        d2 = sb.tile([SG, NG, 9, P], f32)
        nrm = sb.tile([SG, NG, 18], f32)
        rinv = sb.tile([SG, NG, 18], f32)
        qh = sb.tile([SG, NG, 9, 2, C], f32)
        df = sb.tile([SG, NG, 9, C], f32)
        ad = sb.tile([SG, NG, 9, C], f32)
        acc = sb.tile([SG, NG], f32)

        for g in range(NG):
            # d = window - center (rw=4); center cols come out exactly 0
            nc.vector.tensor_tensor(
                out=d[:, g, :, :],
                in0=x[:, g, :, :],
                in1=x[:, g, 4:5, :].to_broadcast([SG, 9, P]),
                op=mybir.AluOpType.subtract,
            )
            nc.scalar.square(out=d2[:, g, :, :], in_=d[:, g, :, :])
            # norm2 over each 64-channel half: view cols as (rw, qk) x 64
            nc.vector.tensor_reduce(
                out=nrm[:, g, :],
                in_=d2[:, g, :, :].rearrange("p rw (t c) -> p (rw t) c", t=2),
                axis=mybir.AxisListType.X,
                op=mybir.AluOpType.add,
            )
            nc.scalar.sqrt(out=nrm[:, g, :], in_=nrm[:, g, :])
            nc.vector.tensor_scalar_add(
                out=nrm[:, g, :], in0=nrm[:, g, :], scalar1=EPS
            )
            nc.vector.reciprocal(out=rinv[:, g, :], in_=nrm[:, g, :])
            # q_hat/k_hat: scale each (rw, half) column group by its rinv
            nc.vector.tensor_tensor(
                out=qh[:, g, :, :, :],
                in0=d[:, g, :, :].rearrange("p rw (t c) -> p (rw t) c", t=2),
                in1=rinv[:, g, :, None].to_broadcast([SG, 18, C]),
                op=mybir.AluOpType.mult,
            )
            nc.vector.tensor_tensor(
                out=df[:, g, :, :],
                in0=qh[:, g, :, 0, :],
                in1=qh[:, g, :, 1, :],
                op=mybir.AluOpType.subtract,
            )
            # |diff| with free-dim accumulate: acc[:, g] = sum |df|
            nc.scalar.activation(
                out=ad[:, g, :, :],
                in_=df[:, g, :, :],
                func=mybir.ActivationFunctionType.Abs,
                accum_out=acc[:, g : g + 1],
            )

        accs = sb.tile([SG, 1], f32)
        nc.vector.tensor_reduce(
            out=accs[:], in_=acc[:], axis=mybir.AxisListType.X, op=mybir.AluOpType.add
        )
        pfin = pf.tile([1, 1], f32, tag="fin")
        nc.tensor.matmul(
            out=pfin[:], lhsT=accs[:], rhs=ones[:], start=True, stop=True
        )
        res = sb.tile([1, 1], f32)
        nc.scalar.copy(out=res[:], in_=pfin[:])
        nc.gpsimd.dma_start(out=out[:], in_=res[:])

    _split_multi_waits(nc)
    return nc


def _make_offsets(ids):
    """offs[s_lo, g*9 + r*3 + w'] = (h_s + r) * W + (w_s + w'), s = g*SG + s_lo."""
    ids = np.asarray(ids, dtype=np.int64)
    h = ids[:, 0]
    w = ids[:, 1]
    r = np.arange(3)
    pix = (h[:, None, None] + r[None, :, None]) * W + (
        w[:, None, None] + r[None, None, :]
    )  # [NUM_S, 3, 3]
    pix = pix.reshape(NUM_S, 9)
    offs = np.empty((SG, NG * 9), dtype=np.uint32)
    for g in range(NG):
        offs[:, g * 9 : (g + 1) * 9] = pix[g * SG : (g + 1) * SG]
    return offs


def kernel(feat_q, feat_k, sample_ids, *, trace=False, trace_cores=None):
    global LAST_RESULTS
    feat_q = np.asarray(feat_q, dtype=np.float32)
    feat_k = np.asarray(feat_k, dtype=np.float32)
    if "nc" not in _cache:
        _cache["nc"] = _build()
    nc = _cache["nc"]

    offs = _make_offsets(sample_ids)
    in_maps = []
    for b in range(N_CORES):
        # [128, H, W] (q stacked on k) -> channel-last [H*W, 128]
        qk = np.concatenate([feat_q[b], feat_k[b]], axis=0)
        qk = np.ascontiguousarray(qk.reshape(P, H * W).T)
        in_maps.append({"fqk": qk, "offs": offs})
    results = run_bass_kernel_spmd(
        nc,
        in_maps,
        core_ids=list(range(N_CORES)),
        trace=trace,
        trace_cores=trace_cores,
    )
    LAST_RESULTS = results
    total = np.float64(0.0)
    for r in results.results:
        total += np.float64(r["out"][0, 0])
    loss = total / (B * C * 8 * NUM_S)
    return np.asarray(loss, dtype=np.float32)
